# revision 1
# baseline (speedup 1.0000x reference)
"""Causal single-head attention on 8 TRN2 NeuronCores — v5.

Problem: x[B=4,T=4096,D=2048] @ Wq/Wk/Wv[D,H=128] -> causal attention -> out[B,T,H].

Sharding (key-parity split, 2 cores per batch): core parity p owns the 16
interleaved 128-row KEY blocks s = 2j+p of its batch and computes K/V
projections only for those; Q is projected for ALL rows (duplicating Q
costs one matrix per core instead of two for K+V, which is what the old
scheme duplicated). Each core computes unnormalized partial attention
(sum exp(s)*v plus the exp-sum denominator) of every query against its
own key half; the host adds the two partials and normalizes
(flash-attention style split-K combine, exact in f32).

Projections run in compensated fp8 (e4m3) with MatmulPerfMode.DoubleRow:
one DoubleRow matmul contracts two 128-d-chunks at 0.5 cycles/row, and
x = x_hi + x_lo, 64*W = w_hi + w_lo are split on the host so
   64*K = x_hi@w_hi + x_hi@w_lo + x_lo@w_hi   (3 DoubleRow ops = 0.75x
bf16 cycles, ~1e-3 rel err; the dropped x_lo@w_lo term is ~1e-3 rel).
The 64 and 1/sqrt(H) factors fold into the exp scale; the host divides
the AV columns by 64 at combine time. Scores and AV stay bf16.

The host permutes each batch's 128-row blocks to [own-parity | other],
so per-core causal structure is core-independent: score chunk k covers
query columns [128k, 2048) (own half, lower-tri mask on its first block)
and [2048+128k, 4096) (other half, first block masked by an all-ones /
all-zeros input depending on parity). V is projected directly in [s,h]
layout (stationary x-chunk, moving w) so no PE transposes are needed.
"""

import numpy as np
import ml_dtypes

B, T, D, H = 4, 4096, 2048, 128
N_CORES = 8
P = 128  # partitions
WSCALE = 64.0  # weight pre-scale so w_lo escapes the fp8 subnormal zone
N_WARM = 16  # PE warmup matmuls (p-state ramp) during the DMA head
N_FILL = 1  # per-tile filler matmuls in group 0

bf16 = ml_dtypes.bfloat16
fp8 = ml_dtypes.float8_e4m3


def build_nc(d=D, tkv=T, h=H):
    import concourse.tile as tile
    from concourse import bacc, mybir

    assert h == P
    n_d = d // P           # 16 d-chunks
    n_pair = n_d // 2      # 8 DoubleRow pairs
    n_ch = 16              # own key chunks (tkv/2/P)
    tq = tkv               # all queries
    n_g = tkv // 512       # 8 qt groups
    half = tkv // 2        # 2048
    scale = 1.0 / (float(np.sqrt(h)) * WSCALE * WSCALE)
    BF = mybir.dt.bfloat16
    F32 = mybir.dt.float32
    FP8 = mybir.dt.float8e4
    DR = mybir.MatmulPerfMode.DoubleRow

    nc = bacc.Bacc("TRN2", target_bir_lowering=False, debug=False,
                   num_devices=N_CORES)

    # x stream: per d-pair rows, free dim = [hi_2d | hi_2d+1 | lo_2d | lo_2d+1]
    # each of length T (so main rhs = [:,0:2,:], lo rhs = [:,2:4,:])
    xf_ext = nc.dram_tensor("xf", [n_pair * P, 4 * tkv], FP8,
                            kind="ExternalInput").ap()
    # weights: [hi-pairs | lo-pairs], each n_pair*2*h (pair-major, chunk, h)
    w_ext = {}
    for nm in ("wq", "wk", "wv"):
        w_ext[nm] = nc.dram_tensor(nm + "f", [P, 2 * n_pair * 2 * h], FP8,
                                   kind="ExternalInput").ap()
    # masks: [tri | other-first-block 0/1]
    mk_ext = nc.dram_tensor("mask2", [P, 2 * P], BF, kind="ExternalInput").ap()
    # partition-major output: row p = t-row within block, 32 blocks of
    # (h+1) cols (own blocks 0-15 then other 16-31) — few big DMA descs
    out_ext = nc.dram_tensor("out", [P, 2 * n_ch * (h + 1)], BF,
                             kind="ExternalOutput").ap()

    xf_r = xf_ext.rearrange("(a p) (four t) -> p a four t", p=P, four=4)

    with tile.TileContext(nc) as tc:
        with (
            tc.tile_pool(name="const", bufs=1) as const_pool,
            tc.tile_pool(name="persist", bufs=1) as persist,
            tc.tile_pool(name="xt", bufs=13) as xt_pool,
            tc.tile_pool(name="outp", bufs=4) as out_pool,
            tc.tile_pool(name="ps512", bufs=2, space="PSUM") as ps512,
            tc.tile_pool(name="pssm", bufs=2, space="PSUM") as pssm,
        ):
            # --- constants ---
            # weights stream in interleaved with the first xt tiles: the hi
            # section of wk first (first K matmul needs only that), the rest
            # behind the early x loads
            w_sb = {}
            for nm in ("wk", "wq", "wv"):
                w_sb[nm] = const_pool.tile([P, 2 * n_pair * 2 * h], FP8,
                                           tag=f"w_{nm}", name=nm)
            sec_b = n_pair * 2 * h
            nc.sync.dma_start(w_sb["wk"][:, 0:sec_b], w_ext["wk"][:, 0:sec_b])
            mk_sb = const_pool.tile([P, 2 * P], BF, tag="mask2")
            tri_m = mk_sb[:, 0:P]
            oth_m = mk_sb[:, P:2 * P]

            def _wdma(nm, sec):
                nc.sync.dma_start(w_sb[nm][:, sec * sec_b:(sec + 1) * sec_b],
                                  w_ext[nm][:, sec * sec_b:(sec + 1) * sec_b])

            def emit_late_consts(pi):
                if pi == 1:
                    _wdma("wq", 0)
                if pi == 3:
                    _wdma("wv", 0)
                if pi == 4:
                    _wdma("wk", 1)
                if pi == 5:
                    _wdma("wq", 1)
                if pi == 6:
                    _wdma("wv", 1)
                    nc.sync.dma_start(mk_sb[:], mk_ext[:])

            def wslice(nm, sec, pi):
                # [P, 2, h] pair slice; sec 0 = hi pairs, 1 = lo pairs
                base = (sec * n_pair + pi) * 2 * h
                return w_sb[nm][:, base:base + 2 * h].rearrange(
                    "p (two m) -> p two m", two=2)

            # --- PE warmup during the DMA-bound head ---
            warm = const_pool.tile([P, 512], BF, tag="warm")
            nc.gpsimd.memset(warm[:], 0.125)
            for _ in range(N_WARM):
                wu_ps = ps512.tile([P, 512], F32, tag="mm512", name="wu_ps")
                nc.tensor.matmul(wu_ps[:, 0:256], warm[:, 0:P],
                                 warm[:, 0:256], start=True, stop=True)

            # --- persistent activations ---
            kt_all = persist.tile([P, half], BF, tag="kt")
            qt_all = persist.tile([P, tq], BF, tag="qt")
            vhat = persist.tile([P, n_ch * (h + 1)], BF, tag="vhat")
            nc.gpsimd.memset(
                vhat[:].rearrange("p (c w) -> p c w", w=h + 1)[:, :, h:h + 1],
                1.0)

            def vh(c):
                return vhat[:, c * (h + 1):(c + 1) * (h + 1)]

            # pt[k]: [own cols 2048-128k | other cols 2048-128k]
            pt = [persist.tile([P, 2 * (half - P * k)], BF, tag=f"pt{k}",
                               name=f"pt{k}")
                  for k in range(n_ch)]

            chunks_ready = 0
            qt_groups_done = []
            scores_done = set()

            def _emit_score(k, g):
                # own half (g<4) or other half (g>=4) segment of chunk k
                if g < 4:
                    base, loc0 = P * k, 0
                else:
                    base, loc0 = half + P * k, half - P * k
                t0 = max(base, 512 * g)
                t1 = 512 * (g + 1)
                if t0 >= t1:
                    return
                w = t1 - t0
                st_ps = ps512.tile([P, w], F32, tag="mm512", name="st_ps")
                nc.tensor.matmul(st_ps[:], kt_all[:, P * k:P * (k + 1)],
                                 qt_all[:, t0:t0 + w], start=True, stop=True)
                lo = loc0 + t0 - base
                nc.scalar.activation(pt[k][:, lo:lo + w], st_ps[:],
                                     mybir.ActivationFunctionType.Exp,
                                     scale=scale)
                if t0 == base:
                    msk = tri_m if g < 4 else oth_m
                    nc.vector.tensor_mul(pt[k][:, loc0:loc0 + P],
                                         pt[k][:, loc0:loc0 + P], msk)

            pending = []  # (kind, thunk); kind 0 = score, 1 = AV run

            def drain(n):
                # pop scores before AV runs (runs' score deps always sit
                # earlier in the queue, so this preserves dependency order
                # while maximizing run-to-exp distance)
                for _ in range(min(n, len(pending))):
                    i = next((j for j, e in enumerate(pending)
                              if e[0] == 0), 0)
                    pending.pop(i)[1]()

            def drain_all():
                # scores first, then AV runs: every run's score deps are
                # already ahead of it, and this maximizes the distance
                # between each run and the exp it waits on; the very last
                # run streams its output in halves to shorten the tail
                pending.sort(key=lambda e: e[0])
                while pending:
                    kind, fn = pending.pop(0)
                    if kind == 1 and not pending:
                        fn(split_out=True)
                    else:
                        fn()

            def flush_scores(cur_g):
                # enqueue newly-available score segments (drained gradually
                # between projection matmuls so ScalarE's exp never bursts);
                # segments of the current group first: the AV runs enqueued
                # right after depend on those, so the rest fill the gap
                new = []
                for k in range(chunks_ready):
                    for g in qt_groups_done:
                        if (k, g) not in scores_done:
                            scores_done.add((k, g))
                            new.append((k, g))
                new.sort(key=lambda kg: (kg[1] != cur_g,) + kg)
                for k, g in new:
                    pending.append(
                        (0, lambda k=k, g=g: _emit_score(k, g)))

            def emit_av(m, other, o_ps, si):
                # accumulate block m into column slice si of the run's PSUM
                # tile; the whole run is DMA'd PSUM->DRAM in one transfer
                for k in range(m + 1):
                    if other:
                        lo = (half - P * k) + P * (m - k)
                    else:
                        lo = P * (m - k)
                    nc.tensor.matmul(
                        o_ps[:, si * (h + 1):(si + 1) * (h + 1)],
                        pt[k][:, lo:lo + P], vh(k),
                        start=(k == 0), stop=(k == m))

            runs_done = set()

            def _emit_av_run(a, other, split_out=False):
                # two half-run PSUM tiles (1 bank each) so the DVE copy of
                # the first half overlaps the second half's AV chains
                stage = out_pool.tile([P, 4 * (h + 1)], BF, tag="osb",
                                      name="o_stage")
                blk0 = 4 * a + (n_ch if other else 0)
                for hf in range(2):
                    o_ps = pssm.tile([P, 2 * (h + 1)], F32, tag="small",
                                     name="o_ps")
                    for si in range(2):
                        emit_av(4 * a + 2 * hf + si, other, o_ps, si)
                    nc.vector.tensor_copy(
                        stage[:, 2 * hf * (h + 1):(2 * hf + 2) * (h + 1)],
                        o_ps[:])
                    if split_out:
                        # last run: half-run DMAs pipeline behind the chains
                        c0, c1 = 2 * hf * (h + 1), (2 * hf + 2) * (h + 1)
                        nc.sync.dma_start(
                            out_ext[:, blk0 * (h + 1) + c0:
                                    blk0 * (h + 1) + c1],
                            stage[:, c0:c1])
                if not split_out:
                    nc.sync.dma_start(
                        out_ext[:, blk0 * (h + 1):(blk0 + 4) * (h + 1)],
                        stage[:])

            def flush_avs():
                # enqueue every 4-block output run whose pt inputs are
                # complete (FIFO after the score segments they depend on)
                for other in (False, True):
                    for a in range(4):
                        g_need = 4 + a if other else a
                        if ((other, a) in runs_done
                                or chunks_ready <= 4 * a + 3
                                or g_need not in qt_groups_done):
                            continue
                        runs_done.add((other, a))
                        pending.append(
                            (1, lambda a=a, other=other, **kw:
                                _emit_av_run(a, other, **kw)))

            # DoubleRow terms: (sec, xs) = (w_hh, x_hh), (w_hh, x_ll),
            # (w_ll, x_hh)
            mains = ((0, 0), (0, 2), (1, 0))

            # --- main loop: own-half groups (K/V/Q + scores) interleaved
            # with other-half groups (Q only) so the exp load on ScalarE is
            # spread across the whole kernel instead of piling up at the end;
            # other groups run descending so the final AV runs are the
            # cheapest (few chunks) ---
            for g in (0, 1, 7, 2, 6, 3, 5, 4):
                own = g < 4
                q_ps = ps512.tile([P, 512], F32, tag="acc", bufs=4, name="q_ps")
                k_ps = (ps512.tile([P, 512], F32, tag="acc", bufs=4,
                                   name="k_ps") if own else None)
                v_ps = (ps512.tile([P, 512], F32, tag="acc", bufs=4,
                                   name="v_ps") if own else None)

                def emit_proj(ps, wname, pi, terms, t0, tn):
                    for ti, (sec, xs) in terms:
                        nc.tensor.matmul(
                            ps[:], wslice(wname, sec, pi),
                            tiles[pi][:, xs:xs + 2, :],
                            start=(pi == 0 and ti == t0),
                            stop=(pi == n_pair - 1 and ti == tn),
                            perf_mode=DR)

                def emit_v_block(i):
                    # one PSUM accumulation group per s-block (groups on
                    # sub-ranges of one tile must not overlap in time: the
                    # psum zero-region is coarser than 512B); hi terms for
                    # all pairs first so the lo weight DMA can trail
                    for pi in range(n_pair):
                        xt3 = tiles[pi]
                        x_hh = xt3[:, 0:2, P * i:P * (i + 1)]
                        x_ll = xt3[:, 2:4, P * i:P * (i + 1)]
                        for ti, (st, mv) in enumerate(
                                ((x_hh, wslice("wv", 0, pi)),
                                 (x_ll, wslice("wv", 0, pi)))):
                            nc.tensor.matmul(
                                v_ps[:, P * i:P * (i + 1)], st, mv,
                                start=(pi == 0 and ti == 0), stop=False,
                                perf_mode=DR)
                    for pi in range(n_pair):
                        xt3 = tiles[pi]
                        x_hh = xt3[:, 0:2, P * i:P * (i + 1)]
                        nc.tensor.matmul(
                            v_ps[:, P * i:P * (i + 1)], x_hh,
                            wslice("wv", 1, pi),
                            start=False, stop=(pi == n_pair - 1),
                            perf_mode=DR)

                tiles = []
                for pi in range(n_pair):
                    # other-half groups: only the x_hi sections are loaded —
                    # their Q drops the x_lo compensation term (score-only
                    # error on half of each row's keys, ~1.1e-2 out rel err,
                    # measured well under the 2e-2 gate) which halves the
                    # other-half DMA stream
                    nsec = 4 if own else 2
                    xt = xt_pool.tile([P, nsec * 512], FP8,
                                      tag="xt" if own else "xto", name="xt")
                    xt3 = xt[:].rearrange("p (four t) -> p four t", four=nsec)
                    # group 0: alternate the issue queue (ScalarE is
                    # idle until the first exps) so DMA issue pipelines
                    # overlap during the cold start
                    eng = nc.scalar if (g == 0 and pi % 2 == 1) else nc.sync
                    eng.dma_start(xt3,
                                  xf_r[:, pi, 0:nsec,
                                       512 * g:512 * (g + 1)])
                    tiles.append(xt3)
                    if g == 0:
                        # head: only the hi-K matmuls run per-tile (they
                        # need just wk_hi); Q and the lo passes follow the
                        # loop so the trailing weight DMAs stay off the
                        # critical path. Fillers soak the DMA-paced slack.
                        emit_late_consts(pi)
                        emit_proj(k_ps, "wk", pi, list(enumerate(mains))[:2],
                                  0, 99)
                        for _ in range(N_FILL):
                            wu = ps512.tile([P, 512], F32, tag="mm512",
                                            name="wu_ps")
                            nc.tensor.matmul(wu[:, 0:256], warm[:, 0:P],
                                             warm[:, 0:256],
                                             start=True, stop=True)
                    else:
                        if own:
                            emit_proj(k_ps, "wk", pi, list(enumerate(mains)),
                                      0, 2)
                            emit_proj(q_ps, "wq", pi, list(enumerate(mains)),
                                      0, 2)
                        else:
                            emit_proj(q_ps, "wq", pi,
                                      [(0, mains[0]), (2, mains[2])], 0, 2)
                        drain(4 if len(pending) > 16 else 2)
                if g == 0:
                    for pi in range(n_pair):
                        emit_proj(k_ps, "wk", pi, list(enumerate(mains))[2:],
                                  99, 2)
                    for pi in range(n_pair):
                        emit_proj(q_ps, "wq", pi, list(enumerate(mains))[:2],
                                  0, 99)
                    for pi in range(n_pair):
                        emit_proj(q_ps, "wq", pi, list(enumerate(mains))[2:],
                                  99, 2)
                nc.vector.tensor_copy(qt_all[:, 512 * g:512 * (g + 1)],
                                      q_ps[:])
                qt_groups_done.append(g)
                if own:
                    nc.vector.tensor_copy(kt_all[:, 512 * g:512 * (g + 1)],
                                          k_ps[:])
                # scores of already-ready chunks against the new qt group
                # enqueue now and drain between the V accumulation blocks
                flush_scores(g)
                if own:
                    for i in range(4):
                        emit_v_block(i)
                        drain(2)
                    for i in range(4):
                        c = 4 * g + i
                        nc.vector.tensor_copy(vh(c)[:, 0:h],
                                              v_ps[:, P * i:P * (i + 1)])
                    chunks_ready = 4 * g + 4
                    flush_scores(g)
                flush_avs()
            drain_all()

    nc.compile()
    return nc


_NC_CACHE = {}


def _get_nc(d=D, tkv=T, h=H):
    key = (d, tkv, h)
    if key not in _NC_CACHE:
        _NC_CACHE[key] = build_nc(d, tkv, h)
    return _NC_CACHE[key]


def _split_fp8(a):
    hi = a.astype(fp8)
    lo = (a - hi.astype(np.float32)).astype(fp8)
    return hi, lo


def make_in_maps(x, Wq, Wk, Wv):
    """Shard full inputs into per-core input maps (host-side prep)."""
    x = np.asarray(x, dtype=np.float32)
    b_, t_, d_ = x.shape
    n_d = d_ // P
    n_pair = n_d // 2
    nb = t_ // P

    def prep_w(w):
        # [P, 2*n_pair*2*h]: [ (hi_2d, hi_2d+1) pairs | (lo_2d, lo_2d+1) ]
        w = np.asarray(w, np.float32) * WSCALE
        hi, lo = _split_fp8(w)
        out = np.empty((P, 2 * n_pair * 2 * H), fp8)
        for sec, src in ((0, hi), (1, lo)):
            # src [D, H] -> chunks [n_d, P, H] -> pair-major layout
            c = src.reshape(n_d, P, H)
            blk = c.transpose(1, 0, 2).reshape(P, n_d * H)
            out[:, sec * n_pair * 2 * H:(sec + 1) * n_pair * 2 * H] = blk
        return np.ascontiguousarray(out)

    wq_f, wk_f, wv_f = prep_w(Wq), prep_w(Wk), prep_w(Wv)
    tri = (np.arange(P)[None, :] >= np.arange(P)[:, None]).astype(bf16)
    ones = np.ones((P, P), dtype=bf16)
    zeros = np.zeros((P, P), dtype=bf16)

    in_maps = []
    for core in range(2 * b_):
        b, p = core // 2, core % 2
        xb = x[b].reshape(nb, P, d_)
        xperm = np.concatenate([xb[p::2], xb[1 - p::2]], axis=0)
        xT = xperm.reshape(t_, d_).T  # [D, T]
        hi, lo = _split_fp8(np.ascontiguousarray(xT))
        # xf [n_pair*P, 4*T]: per pair rows: [hi_2d | hi_2d+1 | lo_2d | lo_2d+1]
        xf = np.empty((n_pair, P, 4, t_), fp8)
        hic = hi.reshape(n_d, P, t_)
        loc = lo.reshape(n_d, P, t_)
        xf[:, :, 0, :] = hic[0::2]
        xf[:, :, 1, :] = hic[1::2]
        xf[:, :, 2, :] = loc[0::2]
        xf[:, :, 3, :] = loc[1::2]
        mask2 = np.concatenate([tri, ones if p == 0 else zeros], axis=1)
        in_maps.append({
            "xf": np.ascontiguousarray(xf.reshape(n_pair * P, 4 * t_)),
            "wqf": wq_f, "wkf": wk_f, "wvf": wv_f,
            "mask2": np.ascontiguousarray(mask2),
        })
    return in_maps


def gather_out(results, b_=B, t_=T, h_=H):
    """Combine per-core unnormalized partials into the full output."""
    nb = t_ // P
    acc = np.zeros((b_, nb, P, h_ + 1), dtype=np.float64)
    for core in range(2 * b_):
        b, p = core // 2, core % 2
        # out [P, nb*(h+1)] -> [nb, P, h+1] (block-major cols, partition=row)
        o = results[core]["out"].astype(np.float64)
        o = o.reshape(P, nb, h_ + 1).transpose(1, 0, 2)
        blocks = np.concatenate([np.arange(p, nb, 2),
                                 np.arange(1 - p, nb, 2)])
        acc[b, blocks] += o
    out = acc[..., :h_] / (acc[..., h_:h_ + 1] * WSCALE)
    return out.reshape(b_, t_, h_).astype(np.float32)


def kernel(x, Wq, Wk, Wv):
    from concourse.bass_utils import run_bass_kernel_spmd

    nc = _get_nc(D, T, H)
    in_maps = make_in_maps(x, Wq, Wk, Wv)
    res = run_bass_kernel_spmd(nc, in_maps, core_ids=list(range(N_CORES)))
    return gather_out(res.results)



# revision 18
# speedup vs baseline: 1.0331x; 1.0331x over previous
"""Causal single-head attention on 8 TRN2 NeuronCores — v6.

Problem: x[B=4,T=4096,D=2048] @ Wq/Wk/Wv[D,H=128] -> causal attention -> out[B,T,H].

Sharding (key-parity split, 2 cores per batch): core parity p owns the 16
interleaved 128-row KEY blocks s = 2j+p of its batch and computes K/V
projections only for those; Q is projected for ALL rows (duplicating Q
costs one matrix per core instead of two for K+V, which is what the old
scheme duplicated). Each core computes unnormalized partial attention
(sum exp(s)*v plus the exp-sum denominator) of every query against its
own key half; the host adds the two partials and normalizes
(flash-attention style split-K combine, exact in f32).

Projections run in compensated fp8 (e4m3) with MatmulPerfMode.DoubleRow:
one DoubleRow matmul contracts two 128-d-chunks at 0.5 cycles/row, and
x = x_hi + x_lo, 64*W = w_hi + w_lo are split on the host so
   64*K = x_hi@w_hi + x_hi@w_lo + x_lo@w_hi   (3 DoubleRow ops = 0.75x
bf16 cycles, ~1e-3 rel err; the dropped x_lo@w_lo term is ~1e-3 rel).
The 64 and 1/sqrt(H) factors fold into the exp scale; the host divides
the AV columns by WSCALE_V at combine time. Scores stay bf16.

v6: the OTHER-half AV (attention-weight @ v) additionally runs in fp8
DoubleRow for key-chunk PAIRS from chunk K0 up: exp writes those chunks'
weights directly as fp8 into pair-interleaved tiles (2 chunks packed for
the 256-deep DoubleRow contraction), and v is stored as a compensated
fp8 hi+lo pair (v scaled by WSCALE_V=16 so it fits e4m3's +-240 range;
v_lo picks up the rounding).  One DR matmul then contracts two key
chunks at 0.5 cyc/row -> other-half AV cost ~halves for those chunks.
Early chunks (< K0) stay bf16: they participate in EVERY query row, so
fp8-ing them noises all rows, while late chunks only touch late rows at
a sub-unity weight fraction (noise scales with sqrt(fraction)).
exp carries bias=ln(1/2) so the fp8 weights stay well under 240 (the
factor cancels between numerator and denominator).  Odd chunks of a
pair are frame-aligned with a memset-zero 128-col head so causal
structure is preserved inside the packed pair.

The host permutes each batch's 128-row blocks to [own-parity | other],
so per-core causal structure is core-independent: score chunk k covers
query columns [128k, 2048) (own half, lower-tri mask on its first block)
and [2048+128k, 4096) (other half, first block masked by an all-ones /
all-zeros input depending on parity). V is projected directly in [s,h]
layout (stationary x-chunk, moving w) so no PE transposes are needed.
"""

import numpy as np
import ml_dtypes

B, T, D, H = 4, 4096, 2048, 128
N_CORES = 8
P = 128  # partitions
WSCALE = 64.0  # weight pre-scale so w_lo escapes the fp8 subnormal zone
WSCALE_V = 16.0  # v pre-scale: v*16 stays under e4m3 max (240)
K_OWN = 4  # first fp8-DR own-half chunk (even); chunks < K_OWN stay bf16
K_OTH = 4  # first fp8-DR other-half chunk (even)
EXP_BIAS = -2.0794415416798357  # ln(1/8): max pt ~81 < e4m3 max 240
N_WARM = 16  # PE warmup matmuls (p-state ramp) during the DMA head
N_FILL = 1  # per-tile filler matmuls in group 0

bf16 = ml_dtypes.bfloat16
fp8 = ml_dtypes.float8_e4m3


def build_nc(d=D, tkv=T, h=H):
    import concourse.tile as tile
    from concourse import bacc, mybir

    assert h == P
    n_d = d // P           # 16 d-chunks
    n_pair = n_d // 2      # 8 DoubleRow pairs
    n_ch = 16              # own key chunks (tkv/2/P)
    tq = tkv               # all queries
    n_g = tkv // 512       # 8 qt groups
    half = tkv // 2        # 2048
    scale = 1.0 / (float(np.sqrt(h)) * WSCALE * WSCALE)
    VW = 144               # padded v-pair frame width (129 -> mult of 16)
    BF = mybir.dt.bfloat16
    F32 = mybir.dt.float32
    FP8 = mybir.dt.float8e4
    DR = mybir.MatmulPerfMode.DoubleRow
    EXP = mybir.ActivationFunctionType.Exp

    nc = bacc.Bacc("TRN2", target_bir_lowering=False, debug=False,
                   num_devices=N_CORES)

    # x stream: per d-pair rows, free dim = [hi_2d | hi_2d+1 | lo_2d | lo_2d+1]
    # each of length T (so main rhs = [:,0:2,:], lo rhs = [:,2:4,:])
    xf_ext = nc.dram_tensor("xf", [n_pair * P, 4 * tkv], FP8,
                            kind="ExternalInput").ap()
    # weights: [hi-pairs | lo-pairs], each n_pair*2*h (pair-major, chunk, h)
    w_ext = {}
    for nm in ("wq", "wk", "wv"):
        w_ext[nm] = nc.dram_tensor(nm + "f", [P, 2 * n_pair * 2 * h], FP8,
                                   kind="ExternalInput").ap()
    # masks: [tri | other-first-block 0/1] bf16, + fp8 copies of both
    mk_ext = nc.dram_tensor("mask2", [P, 2 * P], BF, kind="ExternalInput").ap()
    mk8_ext = nc.dram_tensor("mask8", [P, 2 * P], FP8,
                             kind="ExternalInput").ap()
    # partition-major output: row p = t-row within block, 32 blocks of
    # (h+1) cols (own blocks 0-15 then other 16-31) — few big DMA descs
    out_ext = nc.dram_tensor("out", [P, 2 * n_ch * (h + 1)], BF,
                             kind="ExternalOutput").ap()

    xf_r = xf_ext.rearrange("(a p) (four t) -> p a four t", p=P, four=4)

    with tile.TileContext(nc) as tc:
        with (
            tc.tile_pool(name="const", bufs=1) as const_pool,
            tc.tile_pool(name="persist", bufs=1) as persist,
            tc.tile_pool(name="xt", bufs=13) as xt_pool,
            tc.tile_pool(name="outp", bufs=4) as out_pool,
            tc.tile_pool(name="ps512", bufs=2, space="PSUM") as ps512,
            tc.tile_pool(name="pssm", bufs=2, space="PSUM") as pssm,
        ):
            # --- constants ---
            # weights stream in interleaved with the first xt tiles: the hi
            # section of wk first (first K matmul needs only that), the rest
            # behind the early x loads
            w_sb = {}
            for nm in ("wk", "wq", "wv"):
                w_sb[nm] = const_pool.tile([P, 2 * n_pair * 2 * h], FP8,
                                           tag=f"w_{nm}", name=nm)
            sec_b = n_pair * 2 * h
            nc.sync.dma_start(w_sb["wk"][:, 0:sec_b], w_ext["wk"][:, 0:sec_b])
            mk_sb = const_pool.tile([P, 2 * P], BF, tag="mask2")
            tri_m = mk_sb[:, 0:P]
            oth_m = mk_sb[:, P:2 * P]
            mk8_sb = const_pool.tile([P, 2 * P], FP8, tag="mask8")
            tri_m8 = mk8_sb[:, 0:P]
            oth_m8 = mk8_sb[:, P:2 * P]

            def _wdma(nm, sec):
                nc.sync.dma_start(w_sb[nm][:, sec * sec_b:(sec + 1) * sec_b],
                                  w_ext[nm][:, sec * sec_b:(sec + 1) * sec_b])

            def emit_late_consts(pi):
                if pi == 1:
                    _wdma("wq", 0)
                if pi == 3:
                    _wdma("wv", 0)
                if pi == 4:
                    _wdma("wk", 1)
                if pi == 5:
                    _wdma("wq", 1)
                if pi == 6:
                    _wdma("wv", 1)
                    nc.sync.dma_start(mk_sb[:], mk_ext[:])
                    nc.sync.dma_start(mk8_sb[:], mk8_ext[:])

            def wslice(nm, sec, pi):
                # [P, 2, h] pair slice; sec 0 = hi pairs, 1 = lo pairs
                base = (sec * n_pair + pi) * 2 * h
                return w_sb[nm][:, base:base + 2 * h].rearrange(
                    "p (two m) -> p two m", two=2)

            # --- PE warmup during the DMA-bound head ---
            warm = const_pool.tile([P, 512], BF, tag="warm")
            nc.gpsimd.memset(warm[:], 0.125)
            expb = const_pool.tile([P, 1], F32, tag="expb")
            nc.gpsimd.memset(expb[:], EXP_BIAS)
            for _ in range(N_WARM):
                wu_ps = ps512.tile([P, 512], F32, tag="mm512", name="wu_ps")
                nc.tensor.matmul(wu_ps[:, 0:256], warm[:, 0:P],
                                 warm[:, 0:256], start=True, stop=True)

            # --- persistent activations ---
            kt_all = persist.tile([P, half], BF, tag="kt")
            qt_all = persist.tile([P, tq], BF, tag="qt")
            vhat = persist.tile([P, n_ch * (h + 1)], BF, tag="vhat")
            nc.gpsimd.memset(
                vhat[:].rearrange("p (c w) -> p c w", w=h + 1)[:, :, h:h + 1],
                1.0)
            # fp8 v pairs (hi + lo) for DR chunks: [j][c][VW], col h = denom
            K_MIN = min(K_OWN, K_OTH)
            n_vp = n_ch - K_MIN
            vp_hi = persist.tile([P, n_vp * VW], FP8, tag="vph")
            vp_lo = persist.tile([P, n_vp * VW], FP8, tag="vpl")
            vp_hi3 = vp_hi[:].rearrange("p (jc w) -> p jc w", w=VW)
            vp_lo3 = vp_lo[:].rearrange("p (jc w) -> p jc w", w=VW)
            nc.gpsimd.memset(vp_lo[:], 0.0)
            nc.gpsimd.memset(vp_hi3[:, :, h:h + 1], 1.0)
            # pad cols (h+1..VW) are never read by the 129-wide rhs slices,
            # but memset them anyway so the tile has no uninitialized reads
            nc.gpsimd.memset(vp_hi3[:, :, h + 1:VW], 0.0)

            def vh(c):
                return vhat[:, c * (h + 1):(c + 1) * (h + 1)]

            def vpair(j, lo):
                # [P, 2, h+1] fp8 v pair for chunks (2j, 2j+1)
                jv = 2 * j - K_MIN
                src = vp_lo3 if lo else vp_hi3
                return src[:, jv:jv + 2, 0:h + 1]

            # exp-weight storage per half: chunks < KX in flat bf16 tiles
            # [P, half - 128k]; chunks >= KX in fp8 pair tiles (frame
            # aligned to the even chunk; the odd chunk gets a memset-0
            # 128-col head so causal structure survives the packing)
            pt_flat = {}   # (is_oth, k) -> AP
            pt_pair = {}   # (is_oth, j) -> [P, 2, ow] AP
            for is_oth, kx in ((0, K_OWN), (1, K_OTH)):
                sfx = "o" if is_oth else "w"
                for k in range(kx):
                    pt_flat[is_oth, k] = persist.tile(
                        [P, half - P * k], BF, tag=f"ptf{sfx}{k}",
                        name=f"ptf{sfx}{k}")
                for j in range(kx // 2, n_ch // 2):
                    ow = half - 2 * P * j
                    t_ = persist.tile([P, 2 * ow], FP8, tag=f"ptp{sfx}{j}",
                                      name=f"ptp{sfx}{j}")
                    t3 = t_[:].rearrange("p (two w) -> p two w", two=2)
                    nc.gpsimd.memset(t3[:, 1:2, 0:P], 0.0)
                    pt_pair[is_oth, j] = t3

            chunks_ready = 0
            qt_groups_done = []
            scores_done = set()

            def _emit_score(k, g):
                # own half (g<4) or other half (g>=4) segment of chunk k
                own = g < 4
                is_oth = 0 if own else 1
                kx = K_OWN if own else K_OTH
                base = P * k if own else half + P * k
                t0 = max(base, 512 * g)
                t1 = 512 * (g + 1)
                if t0 >= t1:
                    return
                w = t1 - t0
                st_ps = ps512.tile([P, w], F32, tag="mm512", name="st_ps")
                nc.tensor.matmul(st_ps[:], kt_all[:, P * k:P * (k + 1)],
                                 qt_all[:, t0:t0 + w], start=True, stop=True)
                lo = t0 - base
                if k >= kx:
                    j, c = divmod(k, 2)
                    off = lo + P * c
                    dst = pt_pair[is_oth, j][:, c, off:off + w]
                    msk = tri_m8 if own else oth_m8
                    mdst = pt_pair[is_oth, j][:, c, P * c:P * c + P]
                else:
                    dst = pt_flat[is_oth, k][:, lo:lo + w]
                    msk = tri_m if own else oth_m
                    mdst = pt_flat[is_oth, k][:, 0:P]
                nc.scalar.activation(dst, st_ps[:], EXP,
                                     scale=scale, bias=expb[:])
                if t0 == base:
                    nc.vector.tensor_mul(mdst, mdst, msk)

            pending = []  # (kind, thunk); kind 0 = score, 1 = AV run

            def drain(n):
                # pop scores before AV runs (runs' score deps always sit
                # earlier in the queue, so this preserves dependency order
                # while maximizing run-to-exp distance)
                for _ in range(min(n, len(pending))):
                    i = next((j for j, e in enumerate(pending)
                              if e[0] == 0), 0)
                    pending.pop(i)[1]()

            def drain_all():
                # scores first, then AV runs: every run's score deps are
                # already ahead of it, and this maximizes the distance
                # between each run and the exp it waits on; the very last
                # run streams its output in halves to shorten the tail
                pending.sort(key=lambda e: e[0])
                while pending:
                    kind, fn = pending.pop(0)
                    if kind == 1 and not pending:
                        fn(split_out=True)
                    else:
                        fn()

            def flush_scores(cur_g):
                # enqueue newly-available score segments (drained gradually
                # between projection matmuls so ScalarE's exp never bursts);
                # segments of the current group first: the AV runs enqueued
                # right after depend on those, so the rest fill the gap
                new = []
                for k in range(chunks_ready):
                    for g in qt_groups_done:
                        if (k, g) not in scores_done:
                            scores_done.add((k, g))
                            new.append((k, g))
                new.sort(key=lambda kg: (kg[1] != cur_g,) + kg)
                for k, g in new:
                    pending.append(
                        (0, lambda k=k, g=g: _emit_score(k, g)))

            def emit_av(m, other, o_ps, si):
                # accumulate block m into column slice si of the run's PSUM
                # tile; the whole run is DMA'd PSUM->DRAM in one transfer
                dst = o_ps[:, si * (h + 1):(si + 1) * (h + 1)]
                is_oth = 1 if other else 0
                kx = K_OTH if other else K_OWN
                for k in range(min(m + 1, kx)):
                    nc.tensor.matmul(
                        dst, pt_flat[is_oth, k][:, P * (m - k):
                                                P * (m - k) + P],
                        vh(k), start=(k == 0), stop=(k == m))
                for j in range(kx // 2, m // 2 + 1):
                    off = P * (m - 2 * j)
                    lhs = pt_pair[is_oth, j][:, :, off:off + P]
                    nc.tensor.matmul(
                        dst, lhs, vpair(j, False),
                        start=False, stop=False, perf_mode=DR)
                    nc.tensor.matmul(
                        dst, lhs, vpair(j, True),
                        start=False, stop=(j == m // 2),
                        perf_mode=DR)

            runs_done = set()

            def _emit_av_run(a, other, split_out=False):
                # two half-run PSUM tiles (1 bank each) so the DVE copy of
                # the first half overlaps the second half's AV chains
                stage = out_pool.tile([P, 4 * (h + 1)], BF, tag="osb",
                                      name="o_stage")
                blk0 = 4 * a + (n_ch if other else 0)
                for hf in range(2):
                    o_ps = pssm.tile([P, 2 * (h + 1)], F32, tag="small",
                                     name="o_ps")
                    for si in range(2):
                        emit_av(4 * a + 2 * hf + si, other, o_ps, si)
                    nc.vector.tensor_copy(
                        stage[:, 2 * hf * (h + 1):(2 * hf + 2) * (h + 1)],
                        o_ps[:])
                    if split_out:
                        # last run: half-run DMAs pipeline behind the chains
                        c0, c1 = 2 * hf * (h + 1), (2 * hf + 2) * (h + 1)
                        nc.sync.dma_start(
                            out_ext[:, blk0 * (h + 1) + c0:
                                    blk0 * (h + 1) + c1],
                            stage[:, c0:c1])
                if not split_out:
                    nc.sync.dma_start(
                        out_ext[:, blk0 * (h + 1):(blk0 + 4) * (h + 1)],
                        stage[:])

            def flush_avs():
                # enqueue every 4-block output run whose pt inputs are
                # complete (FIFO after the score segments they depend on)
                for other in (False, True):
                    for a in range(4):
                        g_need = 4 + a if other else a
                        if ((other, a) in runs_done
                                or chunks_ready <= 4 * a + 3
                                or g_need not in qt_groups_done):
                            continue
                        runs_done.add((other, a))
                        pending.append(
                            (1, lambda a=a, other=other, **kw:
                                _emit_av_run(a, other, **kw)))

            # DoubleRow terms: (sec, xs) = (w_hh, x_hh), (w_hh, x_ll),
            # (w_ll, x_hh)
            mains = ((0, 0), (0, 2), (1, 0))

            # --- main loop: own-half groups (K/V/Q + scores) interleaved
            # with other-half groups (Q only) so the exp load on ScalarE is
            # spread across the whole kernel instead of piling up at the end;
            # other groups run descending so the final AV runs are the
            # cheapest (few chunks) ---
            for g in (0, 1, 7, 2, 6, 3, 5, 4):
                own = g < 4
                q_ps = ps512.tile([P, 512], F32, tag="acc", bufs=4, name="q_ps")
                k_ps = (ps512.tile([P, 512], F32, tag="acc", bufs=4,
                                   name="k_ps") if own else None)
                v_ps = (ps512.tile([P, 512], F32, tag="acc", bufs=4,
                                   name="v_ps") if own else None)

                def emit_proj(ps, wname, pi, terms, t0, tn):
                    for ti, (sec, xs) in terms:
                        nc.tensor.matmul(
                            ps[:], wslice(wname, sec, pi),
                            tiles[pi][:, xs:xs + 2, :],
                            start=(pi == 0 and ti == t0),
                            stop=(pi == n_pair - 1 and ti == tn),
                            perf_mode=DR)

                def emit_v_block(i):
                    # one PSUM accumulation group per s-block (groups on
                    # sub-ranges of one tile must not overlap in time: the
                    # psum zero-region is coarser than 512B); hi terms for
                    # all pairs first so the lo weight DMA can trail
                    for pi in range(n_pair):
                        xt3 = tiles[pi]
                        x_hh = xt3[:, 0:2, P * i:P * (i + 1)]
                        x_ll = xt3[:, 2:4, P * i:P * (i + 1)]
                        for ti, (st, mv) in enumerate(
                                ((x_hh, wslice("wv", 0, pi)),
                                 (x_ll, wslice("wv", 0, pi)))):
                            nc.tensor.matmul(
                                v_ps[:, P * i:P * (i + 1)], st, mv,
                                start=(pi == 0 and ti == 0), stop=False,
                                perf_mode=DR)
                    for pi in range(n_pair):
                        xt3 = tiles[pi]
                        x_hh = xt3[:, 0:2, P * i:P * (i + 1)]
                        nc.tensor.matmul(
                            v_ps[:, P * i:P * (i + 1)], x_hh,
                            wslice("wv", 1, pi),
                            start=False, stop=(pi == n_pair - 1),
                            perf_mode=DR)

                tiles = []
                for pi in range(n_pair):
                    # other-half groups: only the x_hi sections are loaded —
                    # their Q drops the x_lo compensation term (score-only
                    # error on half of each row's keys, ~1.1e-2 out rel err,
                    # measured well under the 2e-2 gate) which halves the
                    # other-half DMA stream
                    nsec = 4 if own else 2
                    xt = xt_pool.tile([P, nsec * 512], FP8,
                                      tag="xt" if own else "xto", name="xt")
                    xt3 = xt[:].rearrange("p (four t) -> p four t", four=nsec)
                    # group 0: alternate the issue queue (ScalarE is
                    # idle until the first exps) so DMA issue pipelines
                    # overlap during the cold start
                    eng = nc.scalar if (g == 0 and pi % 2 == 1) else nc.sync
                    eng.dma_start(xt3,
                                  xf_r[:, pi, 0:nsec,
                                       512 * g:512 * (g + 1)])
                    tiles.append(xt3)
                    if g == 0:
                        # head: only the hi-K matmuls run per-tile (they
                        # need just wk_hi); Q and the lo passes follow the
                        # loop so the trailing weight DMAs stay off the
                        # critical path. Fillers soak the DMA-paced slack.
                        emit_late_consts(pi)
                        emit_proj(k_ps, "wk", pi, list(enumerate(mains))[:2],
                                  0, 99)
                        for _ in range(N_FILL):
                            wu = ps512.tile([P, 512], F32, tag="mm512",
                                            name="wu_ps")
                            nc.tensor.matmul(wu[:, 0:256], warm[:, 0:P],
                                             warm[:, 0:256],
                                             start=True, stop=True)
                    else:
                        if own:
                            emit_proj(k_ps, "wk", pi, list(enumerate(mains)),
                                      0, 2)
                            emit_proj(q_ps, "wq", pi, list(enumerate(mains)),
                                      0, 2)
                        else:
                            emit_proj(q_ps, "wq", pi,
                                      [(0, mains[0]), (2, mains[2])], 0, 2)
                        drain(4 if len(pending) > 16 else 2)
                if g == 0:
                    for pi in range(n_pair):
                        emit_proj(k_ps, "wk", pi, list(enumerate(mains))[2:],
                                  99, 2)
                    for pi in range(n_pair):
                        emit_proj(q_ps, "wq", pi, list(enumerate(mains))[:2],
                                  0, 99)
                    for pi in range(n_pair):
                        emit_proj(q_ps, "wq", pi, list(enumerate(mains))[2:],
                                  99, 2)
                nc.vector.tensor_copy(qt_all[:, 512 * g:512 * (g + 1)],
                                      q_ps[:])
                qt_groups_done.append(g)
                if own:
                    nc.vector.tensor_copy(kt_all[:, 512 * g:512 * (g + 1)],
                                          k_ps[:])
                # scores of already-ready chunks against the new qt group
                # enqueue now and drain between the V accumulation blocks
                flush_scores(g)
                if own:
                    for i in range(4):
                        emit_v_block(i)
                        drain(2)
                    for i in range(4):
                        c = 4 * g + i
                        nc.vector.tensor_copy(vh(c)[:, 0:h],
                                              v_ps[:, P * i:P * (i + 1)])
                        if c >= K_MIN:
                            jc = c - K_MIN  # index into the pair-frame axis
                            nc.vector.tensor_copy(
                                vp_hi3[:, jc, 0:h],
                                v_ps[:, P * i:P * (i + 1)])
                            nc.vector.tensor_sub(
                                vp_lo3[:, jc, 0:h],
                                v_ps[:, P * i:P * (i + 1)],
                                vp_hi3[:, jc, 0:h])
                    chunks_ready = 4 * g + 4
                    flush_scores(g)
                flush_avs()
            drain_all()

    nc.compile()
    return nc


_NC_CACHE = {}


def _get_nc(d=D, tkv=T, h=H):
    key = (d, tkv, h)
    if key not in _NC_CACHE:
        _NC_CACHE[key] = build_nc(d, tkv, h)
    return _NC_CACHE[key]


def _split_fp8(a):
    hi = a.astype(fp8)
    lo = (a - hi.astype(np.float32)).astype(fp8)
    return hi, lo


def make_in_maps(x, Wq, Wk, Wv):
    """Shard full inputs into per-core input maps (host-side prep)."""
    x = np.asarray(x, dtype=np.float32)
    b_, t_, d_ = x.shape
    n_d = d_ // P
    n_pair = n_d // 2
    nb = t_ // P

    def prep_w(w, ws):
        # [P, 2*n_pair*2*h]: [ (hi_2d, hi_2d+1) pairs | (lo_2d, lo_2d+1) ]
        w = np.asarray(w, np.float32) * ws
        hi, lo = _split_fp8(w)
        out = np.empty((P, 2 * n_pair * 2 * H), fp8)
        for sec, src in ((0, hi), (1, lo)):
            # src [D, H] -> chunks [n_d, P, H] -> pair-major layout
            c = src.reshape(n_d, P, H)
            blk = c.transpose(1, 0, 2).reshape(P, n_d * H)
            out[:, sec * n_pair * 2 * H:(sec + 1) * n_pair * 2 * H] = blk
        return np.ascontiguousarray(out)

    wq_f = prep_w(Wq, WSCALE)
    wk_f = prep_w(Wk, WSCALE)
    wv_f = prep_w(Wv, WSCALE_V)
    tri = (np.arange(P)[None, :] >= np.arange(P)[:, None]).astype(bf16)
    ones = np.ones((P, P), dtype=bf16)
    zeros = np.zeros((P, P), dtype=bf16)

    in_maps = []
    for core in range(2 * b_):
        b, p = core // 2, core % 2
        xb = x[b].reshape(nb, P, d_)
        xperm = np.concatenate([xb[p::2], xb[1 - p::2]], axis=0)
        xT = xperm.reshape(t_, d_).T  # [D, T]
        hi, lo = _split_fp8(np.ascontiguousarray(xT))
        # xf [n_pair*P, 4*T]: per pair rows: [hi_2d | hi_2d+1 | lo_2d | lo_2d+1]
        xf = np.empty((n_pair, P, 4, t_), fp8)
        hic = hi.reshape(n_d, P, t_)
        loc = lo.reshape(n_d, P, t_)
        xf[:, :, 0, :] = hic[0::2]
        xf[:, :, 1, :] = hic[1::2]
        xf[:, :, 2, :] = loc[0::2]
        xf[:, :, 3, :] = loc[1::2]
        oth = ones if p == 0 else zeros
        mask2 = np.concatenate([tri, oth], axis=1)
        in_maps.append({
            "xf": np.ascontiguousarray(xf.reshape(n_pair * P, 4 * t_)),
            "wqf": wq_f, "wkf": wk_f, "wvf": wv_f,
            "mask2": np.ascontiguousarray(mask2),
            "mask8": np.ascontiguousarray(mask2.astype(fp8)),
        })
    return in_maps


def gather_out(results, b_=B, t_=T, h_=H):
    """Combine per-core unnormalized partials into the full output."""
    nb = t_ // P
    acc = np.zeros((b_, nb, P, h_ + 1), dtype=np.float64)
    for core in range(2 * b_):
        b, p = core // 2, core % 2
        # out [P, nb*(h+1)] -> [nb, P, h+1] (block-major cols, partition=row)
        o = results[core]["out"].astype(np.float64)
        o = o.reshape(P, nb, h_ + 1).transpose(1, 0, 2)
        blocks = np.concatenate([np.arange(p, nb, 2),
                                 np.arange(1 - p, nb, 2)])
        acc[b, blocks] += o
    out = acc[..., :h_] / (acc[..., h_:h_ + 1] * WSCALE_V)
    return out.reshape(b_, t_, h_).astype(np.float32)


def kernel(x, Wq, Wk, Wv):
    from concourse.bass_utils import run_bass_kernel_spmd

    nc = _get_nc(D, T, H)
    in_maps = make_in_maps(x, Wq, Wk, Wv)
    res = run_bass_kernel_spmd(nc, in_maps, core_ids=list(range(N_CORES)))
    return gather_out(res.results)


# revision 26
# speedup vs baseline: 1.0427x; 1.0093x over previous
"""Causal single-head attention on 8 TRN2 NeuronCores — v6.

Problem: x[B=4,T=4096,D=2048] @ Wq/Wk/Wv[D,H=128] -> causal attention -> out[B,T,H].

Sharding (key-parity split, 2 cores per batch): core parity p owns the 16
interleaved 128-row KEY blocks s = 2j+p of its batch and computes K/V
projections only for those; Q is projected for ALL rows (duplicating Q
costs one matrix per core instead of two for K+V, which is what the old
scheme duplicated). Each core computes unnormalized partial attention
(sum exp(s)*v plus the exp-sum denominator) of every query against its
own key half; the host adds the two partials and normalizes
(flash-attention style split-K combine, exact in f32).

Projections run in compensated fp8 (e4m3) with MatmulPerfMode.DoubleRow:
one DoubleRow matmul contracts two 128-d-chunks at 0.5 cycles/row, and
x = x_hi + x_lo, 64*W = w_hi + w_lo are split on the host so
   64*K = x_hi@w_hi + x_hi@w_lo + x_lo@w_hi   (3 DoubleRow ops = 0.75x
bf16 cycles, ~1e-3 rel err; the dropped x_lo@w_lo term is ~1e-3 rel).
The 64 and 1/sqrt(H) factors fold into the exp scale; the host divides
the AV columns by WSCALE_V at combine time. Scores stay bf16.

v6: the OTHER-half AV (attention-weight @ v) additionally runs in fp8
DoubleRow for key-chunk PAIRS from chunk K0 up: exp writes those chunks'
weights directly as fp8 into pair-interleaved tiles (2 chunks packed for
the 256-deep DoubleRow contraction), and v is stored as a compensated
fp8 hi+lo pair (v scaled by WSCALE_V=16 so it fits e4m3's +-240 range;
v_lo picks up the rounding).  One DR matmul then contracts two key
chunks at 0.5 cyc/row -> other-half AV cost ~halves for those chunks.
Early chunks (< K0) stay bf16: they participate in EVERY query row, so
fp8-ing them noises all rows, while late chunks only touch late rows at
a sub-unity weight fraction (noise scales with sqrt(fraction)).
exp carries bias=ln(1/2) so the fp8 weights stay well under 240 (the
factor cancels between numerator and denominator).  Odd chunks of a
pair are frame-aligned with a memset-zero 128-col head so causal
structure is preserved inside the packed pair.

The host permutes each batch's 128-row blocks to [own-parity | other],
so per-core causal structure is core-independent: score chunk k covers
query columns [128k, 2048) (own half, lower-tri mask on its first block)
and [2048+128k, 4096) (other half, first block masked by an all-ones /
all-zeros input depending on parity). V is projected directly in [s,h]
layout (stationary x-chunk, moving w) so no PE transposes are needed.
"""

import numpy as np
import ml_dtypes

B, T, D, H = 4, 4096, 2048, 128
N_CORES = 8
P = 128  # partitions
WSCALE = 64.0  # weight pre-scale so w_lo escapes the fp8 subnormal zone
WSCALE_V = 16.0  # v pre-scale: v*16 stays under e4m3 max (240)
K_OWN = 2  # first fp8-DR own-half chunk (even); chunks < K_OWN stay bf16
K_OTH = 2  # first fp8-DR other-half chunk (even)
EXP_BIAS = -2.0794415416798357  # ln(1/8): max pt ~81 < e4m3 max 240
N_WARM = 16  # PE warmup matmuls (p-state ramp) during the DMA head
N_FILL = 1  # per-tile filler matmuls in group 0

bf16 = ml_dtypes.bfloat16
fp8 = ml_dtypes.float8_e4m3


def build_nc(d=D, tkv=T, h=H):
    import concourse.tile as tile
    from concourse import bacc, mybir

    assert h == P
    n_d = d // P           # 16 d-chunks
    n_pair = n_d // 2      # 8 DoubleRow pairs
    n_ch = 16              # own key chunks (tkv/2/P)
    tq = tkv               # all queries
    n_g = tkv // 512       # 8 qt groups
    half = tkv // 2        # 2048
    scale = 1.0 / (float(np.sqrt(h)) * WSCALE * WSCALE)
    VW = 144               # padded v-pair frame width (129 -> mult of 16)
    BF = mybir.dt.bfloat16
    F32 = mybir.dt.float32
    FP8 = mybir.dt.float8e4
    DR = mybir.MatmulPerfMode.DoubleRow
    EXP = mybir.ActivationFunctionType.Exp

    nc = bacc.Bacc("TRN2", target_bir_lowering=False, debug=False,
                   num_devices=N_CORES)

    # x stream: per d-pair rows, free dim = [hi_2d | hi_2d+1 | lo_2d | lo_2d+1]
    # each of length T (so main rhs = [:,0:2,:], lo rhs = [:,2:4,:])
    xf_ext = nc.dram_tensor("xf", [n_pair * P, 4 * tkv], FP8,
                            kind="ExternalInput").ap()
    # weights: [hi-pairs | lo-pairs], each n_pair*2*h (pair-major, chunk, h)
    w_ext = {}
    for nm in ("wq", "wk", "wv"):
        w_ext[nm] = nc.dram_tensor(nm + "f", [P, 2 * n_pair * 2 * h], FP8,
                                   kind="ExternalInput").ap()
    # masks: [tri | other-first-block 0/1] bf16, + fp8 copies of both
    mk_ext = nc.dram_tensor("mask2", [P, 2 * P], BF, kind="ExternalInput").ap()
    mk8_ext = nc.dram_tensor("mask8", [P, 2 * P], FP8,
                             kind="ExternalInput").ap()
    # partition-major output: row p = t-row within block, 32 blocks of
    # (h+1) cols (own blocks 0-15 then other 16-31) — few big DMA descs
    out_ext = nc.dram_tensor("out", [P, 2 * n_ch * (h + 1)], BF,
                             kind="ExternalOutput").ap()

    xf_r = xf_ext.rearrange("(a p) (four t) -> p a four t", p=P, four=4)

    with tile.TileContext(nc) as tc:
        with (
            tc.tile_pool(name="const", bufs=1) as const_pool,
            tc.tile_pool(name="persist", bufs=1) as persist,
            tc.tile_pool(name="xt", bufs=13) as xt_pool,
            tc.tile_pool(name="outp", bufs=4) as out_pool,
            tc.tile_pool(name="ps512", bufs=2, space="PSUM") as ps512,
            tc.tile_pool(name="pssm", bufs=2, space="PSUM") as pssm,
        ):
            # --- constants ---
            # weights stream in interleaved with the first xt tiles: the hi
            # section of wk first (first K matmul needs only that), the rest
            # behind the early x loads
            w_sb = {}
            for nm in ("wk", "wq", "wv"):
                w_sb[nm] = const_pool.tile([P, 2 * n_pair * 2 * h], FP8,
                                           tag=f"w_{nm}", name=nm)
            sec_b = n_pair * 2 * h
            nc.sync.dma_start(w_sb["wk"][:, 0:sec_b], w_ext["wk"][:, 0:sec_b])
            mk_sb = const_pool.tile([P, 2 * P], BF, tag="mask2")
            tri_m = mk_sb[:, 0:P]
            oth_m = mk_sb[:, P:2 * P]
            mk8_sb = const_pool.tile([P, 2 * P], FP8, tag="mask8")
            tri_m8 = mk8_sb[:, 0:P]
            oth_m8 = mk8_sb[:, P:2 * P]

            def _wdma(nm, sec):
                nc.sync.dma_start(w_sb[nm][:, sec * sec_b:(sec + 1) * sec_b],
                                  w_ext[nm][:, sec * sec_b:(sec + 1) * sec_b])

            def emit_late_consts(pi):
                if pi == 1:
                    _wdma("wq", 0)
                if pi == 3:
                    _wdma("wv", 0)
                if pi == 4:
                    _wdma("wk", 1)
                if pi == 5:
                    _wdma("wq", 1)
                if pi == 6:
                    _wdma("wv", 1)
                    nc.sync.dma_start(mk_sb[:], mk_ext[:])
                    nc.sync.dma_start(mk8_sb[:], mk8_ext[:])

            def wslice(nm, sec, pi):
                # [P, 2, h] pair slice; sec 0 = hi pairs, 1 = lo pairs
                base = (sec * n_pair + pi) * 2 * h
                return w_sb[nm][:, base:base + 2 * h].rearrange(
                    "p (two m) -> p two m", two=2)

            # --- PE warmup during the DMA-bound head ---
            warm = const_pool.tile([P, 512], BF, tag="warm")
            nc.gpsimd.memset(warm[:], 0.125)
            expb = const_pool.tile([P, 1], F32, tag="expb")
            nc.gpsimd.memset(expb[:], EXP_BIAS)
            for _ in range(N_WARM):
                wu_ps = ps512.tile([P, 512], F32, tag="mm512", name="wu_ps")
                nc.tensor.matmul(wu_ps[:, 0:256], warm[:, 0:P],
                                 warm[:, 0:256], start=True, stop=True)

            # --- persistent activations ---
            kt_all = persist.tile([P, half], BF, tag="kt")
            qt_all = persist.tile([P, tq], BF, tag="qt")
            vhat = persist.tile([P, n_ch * (h + 1)], BF, tag="vhat")
            nc.gpsimd.memset(
                vhat[:].rearrange("p (c w) -> p c w", w=h + 1)[:, :, h:h + 1],
                1.0)
            # fp8 v pairs (hi + lo) for DR chunks: [j][c][VW], col h = denom
            K_MIN = min(K_OWN, K_OTH)
            n_vp = n_ch - K_MIN
            vp_hi = persist.tile([P, n_vp * VW], FP8, tag="vph")
            vp_lo = persist.tile([P, n_vp * VW], FP8, tag="vpl")
            vp_hi3 = vp_hi[:].rearrange("p (jc w) -> p jc w", w=VW)
            vp_lo3 = vp_lo[:].rearrange("p (jc w) -> p jc w", w=VW)
            nc.gpsimd.memset(vp_lo[:], 0.0)
            nc.gpsimd.memset(vp_hi3[:, :, h:h + 1], 1.0)
            # pad cols (h+1..VW) are never read by the 129-wide rhs slices,
            # but memset them anyway so the tile has no uninitialized reads
            nc.gpsimd.memset(vp_hi3[:, :, h + 1:VW], 0.0)

            def vh(c):
                return vhat[:, c * (h + 1):(c + 1) * (h + 1)]

            def vpair(j, lo):
                # [P, 2, h+1] fp8 v pair for chunks (2j, 2j+1)
                jv = 2 * j - K_MIN
                src = vp_lo3 if lo else vp_hi3
                return src[:, jv:jv + 2, 0:h + 1]

            # exp-weight storage per half: chunks < KX in flat bf16 tiles
            # [P, half - 128k]; chunks >= KX in fp8 pair tiles (frame
            # aligned to the even chunk; the odd chunk gets a memset-0
            # 128-col head so causal structure survives the packing)
            pt_flat = {}   # (is_oth, k) -> AP
            pt_pair = {}   # (is_oth, j) -> [P, 2, ow] AP
            for is_oth, kx in ((0, K_OWN), (1, K_OTH)):
                sfx = "o" if is_oth else "w"
                for k in range(kx):
                    pt_flat[is_oth, k] = persist.tile(
                        [P, half - P * k], BF, tag=f"ptf{sfx}{k}",
                        name=f"ptf{sfx}{k}")
                for j in range(kx // 2, n_ch // 2):
                    ow = half - 2 * P * j
                    t_ = persist.tile([P, 2 * ow], FP8, tag=f"ptp{sfx}{j}",
                                      name=f"ptp{sfx}{j}")
                    t3 = t_[:].rearrange("p (two w) -> p two w", two=2)
                    nc.gpsimd.memset(t3[:, 1:2, 0:P], 0.0)
                    pt_pair[is_oth, j] = t3

            chunks_ready = 0
            qt_groups_done = []
            scores_done = set()

            def _emit_score(k, g):
                # own half (g<4) or other half (g>=4) segment of chunk k
                own = g < 4
                is_oth = 0 if own else 1
                kx = K_OWN if own else K_OTH
                base = P * k if own else half + P * k
                t0 = max(base, 512 * g)
                t1 = 512 * (g + 1)
                if t0 >= t1:
                    return
                w = t1 - t0
                st_ps = ps512.tile([P, w], F32, tag="mm512", name="st_ps")
                nc.tensor.matmul(st_ps[:], kt_all[:, P * k:P * (k + 1)],
                                 qt_all[:, t0:t0 + w], start=True, stop=True)
                lo = t0 - base
                if k >= kx:
                    j, c = divmod(k, 2)
                    off = lo + P * c
                    dst = pt_pair[is_oth, j][:, c, off:off + w]
                    msk = tri_m8 if own else oth_m8
                    mdst = pt_pair[is_oth, j][:, c, P * c:P * c + P]
                else:
                    dst = pt_flat[is_oth, k][:, lo:lo + w]
                    msk = tri_m if own else oth_m
                    mdst = pt_flat[is_oth, k][:, 0:P]
                # the terminal group's exps gate the very last AV run:
                # stream them in 256-col pieces so its matmuls start sooner
                pw = 256 if (not own and g == 4 and w > 256) else w
                for p0 in range(0, w, pw):
                    pend = min(w, p0 + pw)
                    nc.scalar.activation(dst[:, p0:pend],
                                         st_ps[:, p0:pend], EXP,
                                         scale=scale, bias=expb[:])
                    if t0 == base and p0 == 0:
                        nc.vector.tensor_mul(mdst, mdst, msk)

            # (kind, thunk); kind 0 = score, 1 = AV run, 2 = deferred AV
            # run (early deps, held back to fill PE while the terminal
            # exps grind on ScalarE)
            pending = []

            def drain(n):
                # pop scores before AV runs (runs' score deps always sit
                # earlier in the queue, so this preserves dependency order
                # while maximizing run-to-exp distance); kind-2 runs are
                # reserved for the terminal drain
                for _ in range(min(n, len(pending))):
                    i = next((j for j, e in enumerate(pending)
                              if e[0] == 0), None)
                    if i is None:
                        i = next((j for j, e in enumerate(pending)
                                  if e[0] == 1), None)
                    if i is None:
                        return
                    pending.pop(i)[1]()

            def drain_all():
                # scores first, then the deferred runs (their exps are long
                # done, so they execute while ScalarE works the terminal
                # exps), then the late-dep runs; the very last run streams
                # its output per-block to shorten the tail
                pending.sort(key=lambda e: {0: 0, 2: 1, 1: 2}[e[0]])
                while pending:
                    kind, fn = pending.pop(0)
                    if kind == 1 and not pending:
                        fn(split_out=True)
                    else:
                        fn()

            def flush_scores(cur_g):
                # enqueue newly-available score segments (drained gradually
                # between projection matmuls so ScalarE's exp never bursts);
                # segments of the current group first: the AV runs enqueued
                # right after depend on those, so the rest fill the gap
                new = []
                for k in range(chunks_ready):
                    for g in qt_groups_done:
                        if (k, g) not in scores_done:
                            scores_done.add((k, g))
                            new.append((k, g))
                new.sort(key=lambda kg: (kg[1] != cur_g,) + kg)
                for k, g in new:
                    pending.append(
                        (0, lambda k=k, g=g: _emit_score(k, g)))

            def emit_av(m, other, o_ps, si):
                # accumulate block m into column slice si of the run's PSUM
                # tile; the whole run is DMA'd PSUM->DRAM in one transfer
                dst = o_ps[:, si * (h + 1):(si + 1) * (h + 1)]
                is_oth = 1 if other else 0
                kx = K_OTH if other else K_OWN
                for k in range(min(m + 1, kx)):
                    nc.tensor.matmul(
                        dst, pt_flat[is_oth, k][:, P * (m - k):
                                                P * (m - k) + P],
                        vh(k), start=(k == 0), stop=(k == m))
                for j in range(kx // 2, m // 2 + 1):
                    off = P * (m - 2 * j)
                    lhs = pt_pair[is_oth, j][:, :, off:off + P]
                    nc.tensor.matmul(
                        dst, lhs, vpair(j, False),
                        start=False, stop=False, perf_mode=DR)
                    nc.tensor.matmul(
                        dst, lhs, vpair(j, True),
                        start=False, stop=(j == m // 2),
                        perf_mode=DR)

            runs_done = set()

            def _emit_av_run(a, other, split_out=False):
                # two half-run PSUM tiles (1 bank each) so the DVE copy of
                # the first half overlaps the second half's AV chains
                stage = out_pool.tile([P, 4 * (h + 1)], BF, tag="osb",
                                      name="o_stage")
                blk0 = 4 * a + (n_ch if other else 0)
                for hf in range(2):
                    o_ps = pssm.tile([P, 2 * (h + 1)], F32, tag="small",
                                     name="o_ps")
                    for si in range(2):
                        emit_av(4 * a + 2 * hf + si, other, o_ps, si)
                        if split_out:
                            # last run: per-block copy+DMA pipeline behind
                            # the remaining AV chains to shorten the tail
                            cc = (2 * hf + si) * (h + 1)
                            nc.vector.tensor_copy(
                                stage[:, cc:cc + h + 1],
                                o_ps[:, si * (h + 1):(si + 1) * (h + 1)])
                            nc.sync.dma_start(
                                out_ext[:, blk0 * (h + 1) + cc:
                                        blk0 * (h + 1) + cc + h + 1],
                                stage[:, cc:cc + h + 1])
                    if not split_out:
                        nc.vector.tensor_copy(
                            stage[:, 2 * hf * (h + 1):
                                  (2 * hf + 2) * (h + 1)],
                            o_ps[:])
                if not split_out:
                    nc.sync.dma_start(
                        out_ext[:, blk0 * (h + 1):(blk0 + 4) * (h + 1)],
                        stage[:])

            def flush_avs():
                # enqueue every 4-block output run whose pt inputs are
                # complete (FIFO after the score segments they depend on)
                for other in (False, True):
                    for a in range(4):
                        g_need = 4 + a if other else a
                        if ((other, a) in runs_done
                                or chunks_ready <= 4 * a + 3
                                or g_need not in qt_groups_done):
                            continue
                        runs_done.add((other, a))
                        pending.append(
                            (1, lambda a=a, other=other, **kw:
                                _emit_av_run(a, other, **kw)))

            # DoubleRow terms: (sec, xs) = (w_hh, x_hh), (w_hh, x_ll),
            # (w_ll, x_hh)
            mains = ((0, 0), (0, 2), (1, 0))

            # --- main loop: own-half groups (K/V/Q + scores) interleaved
            # with other-half groups (Q only) so the exp load on ScalarE is
            # spread across the whole kernel instead of piling up at the end;
            # other groups run descending so the final AV runs are the
            # cheapest (few chunks) ---
            for g in (0, 1, 7, 2, 6, 3, 5, 4):
                own = g < 4
                q_ps = ps512.tile([P, 512], F32, tag="acc", bufs=4, name="q_ps")
                k_ps = (ps512.tile([P, 512], F32, tag="acc", bufs=4,
                                   name="k_ps") if own else None)
                v_ps = (ps512.tile([P, 512], F32, tag="acc", bufs=4,
                                   name="v_ps") if own else None)

                def emit_proj(ps, wname, pi, terms, t0, tn):
                    for ti, (sec, xs) in terms:
                        nc.tensor.matmul(
                            ps[:], wslice(wname, sec, pi),
                            tiles[pi][:, xs:xs + 2, :],
                            start=(pi == 0 and ti == t0),
                            stop=(pi == n_pair - 1 and ti == tn),
                            perf_mode=DR)

                def emit_v_block(i):
                    # one PSUM accumulation group per s-block (groups on
                    # sub-ranges of one tile must not overlap in time: the
                    # psum zero-region is coarser than 512B); hi terms for
                    # all pairs first so the lo weight DMA can trail
                    for pi in range(n_pair):
                        xt3 = tiles[pi]
                        x_hh = xt3[:, 0:2, P * i:P * (i + 1)]
                        x_ll = xt3[:, 2:4, P * i:P * (i + 1)]
                        for ti, (st, mv) in enumerate(
                                ((x_hh, wslice("wv", 0, pi)),
                                 (x_ll, wslice("wv", 0, pi)))):
                            nc.tensor.matmul(
                                v_ps[:, P * i:P * (i + 1)], st, mv,
                                start=(pi == 0 and ti == 0), stop=False,
                                perf_mode=DR)
                    for pi in range(n_pair):
                        xt3 = tiles[pi]
                        x_hh = xt3[:, 0:2, P * i:P * (i + 1)]
                        nc.tensor.matmul(
                            v_ps[:, P * i:P * (i + 1)], x_hh,
                            wslice("wv", 1, pi),
                            start=False, stop=(pi == n_pair - 1),
                            perf_mode=DR)

                tiles = []
                for pi in range(n_pair):
                    # other-half groups: only the x_hi sections are loaded —
                    # their Q drops the x_lo compensation term (score-only
                    # error on half of each row's keys, ~1.1e-2 out rel err,
                    # measured well under the 2e-2 gate) which halves the
                    # other-half DMA stream
                    nsec = 4 if own else 2
                    xt = xt_pool.tile([P, nsec * 512], FP8,
                                      tag="xt" if own else "xto", name="xt")
                    xt3 = xt[:].rearrange("p (four t) -> p four t", four=nsec)
                    # group 0: alternate the issue queue (ScalarE is
                    # idle until the first exps) so DMA issue pipelines
                    # overlap during the cold start
                    eng = nc.scalar if (g == 0 and pi % 2 == 1) else nc.sync
                    eng.dma_start(xt3,
                                  xf_r[:, pi, 0:nsec,
                                       512 * g:512 * (g + 1)])
                    tiles.append(xt3)
                    if g == 0:
                        # head: only the hi-K matmuls run per-tile (they
                        # need just wk_hi); Q and the lo passes follow the
                        # loop so the trailing weight DMAs stay off the
                        # critical path. Fillers soak the DMA-paced slack.
                        emit_late_consts(pi)
                        emit_proj(k_ps, "wk", pi, list(enumerate(mains))[:2],
                                  0, 99)
                        for _ in range(N_FILL):
                            wu = ps512.tile([P, 512], F32, tag="mm512",
                                            name="wu_ps")
                            nc.tensor.matmul(wu[:, 0:256], warm[:, 0:P],
                                             warm[:, 0:256],
                                             start=True, stop=True)
                    else:
                        if own:
                            emit_proj(k_ps, "wk", pi, list(enumerate(mains)),
                                      0, 2)
                            emit_proj(q_ps, "wq", pi, list(enumerate(mains)),
                                      0, 2)
                        else:
                            emit_proj(q_ps, "wq", pi,
                                      [(0, mains[0]), (2, mains[2])], 0, 2)
                        drain(4 if len(pending) > 16 else 2)
                if g == 0:
                    for pi in range(n_pair):
                        emit_proj(k_ps, "wk", pi, list(enumerate(mains))[2:],
                                  99, 2)
                    for pi in range(n_pair):
                        emit_proj(q_ps, "wq", pi, list(enumerate(mains))[:2],
                                  0, 99)
                    for pi in range(n_pair):
                        emit_proj(q_ps, "wq", pi, list(enumerate(mains))[2:],
                                  99, 2)
                nc.vector.tensor_copy(qt_all[:, 512 * g:512 * (g + 1)],
                                      q_ps[:])
                qt_groups_done.append(g)
                if own:
                    nc.vector.tensor_copy(kt_all[:, 512 * g:512 * (g + 1)],
                                          k_ps[:])
                # scores of already-ready chunks against the new qt group
                # enqueue now and drain between the V accumulation blocks
                flush_scores(g)
                if own:
                    for i in range(4):
                        emit_v_block(i)
                        drain(2)
                    for i in range(4):
                        c = 4 * g + i
                        nc.vector.tensor_copy(vh(c)[:, 0:h],
                                              v_ps[:, P * i:P * (i + 1)])
                        if c >= K_MIN:
                            jc = c - K_MIN  # index into the pair-frame axis
                            nc.vector.tensor_copy(
                                vp_hi3[:, jc, 0:h],
                                v_ps[:, P * i:P * (i + 1)])
                            nc.vector.tensor_sub(
                                vp_lo3[:, jc, 0:h],
                                v_ps[:, P * i:P * (i + 1)],
                                vp_hi3[:, jc, 0:h])
                    chunks_ready = 4 * g + 4
                    flush_scores(g)
                flush_avs()
            drain_all()

    nc.compile()
    return nc


_NC_CACHE = {}


def _get_nc(d=D, tkv=T, h=H):
    key = (d, tkv, h)
    if key not in _NC_CACHE:
        _NC_CACHE[key] = build_nc(d, tkv, h)
    return _NC_CACHE[key]


def _split_fp8(a):
    hi = a.astype(fp8)
    lo = (a - hi.astype(np.float32)).astype(fp8)
    return hi, lo


def make_in_maps(x, Wq, Wk, Wv):
    """Shard full inputs into per-core input maps (host-side prep)."""
    x = np.asarray(x, dtype=np.float32)
    b_, t_, d_ = x.shape
    n_d = d_ // P
    n_pair = n_d // 2
    nb = t_ // P

    def prep_w(w, ws):
        # [P, 2*n_pair*2*h]: [ (hi_2d, hi_2d+1) pairs | (lo_2d, lo_2d+1) ]
        w = np.asarray(w, np.float32) * ws
        hi, lo = _split_fp8(w)
        out = np.empty((P, 2 * n_pair * 2 * H), fp8)
        for sec, src in ((0, hi), (1, lo)):
            # src [D, H] -> chunks [n_d, P, H] -> pair-major layout
            c = src.reshape(n_d, P, H)
            blk = c.transpose(1, 0, 2).reshape(P, n_d * H)
            out[:, sec * n_pair * 2 * H:(sec + 1) * n_pair * 2 * H] = blk
        return np.ascontiguousarray(out)

    wq_f = prep_w(Wq, WSCALE)
    wk_f = prep_w(Wk, WSCALE)
    wv_f = prep_w(Wv, WSCALE_V)
    tri = (np.arange(P)[None, :] >= np.arange(P)[:, None]).astype(bf16)
    ones = np.ones((P, P), dtype=bf16)
    zeros = np.zeros((P, P), dtype=bf16)

    in_maps = []
    for core in range(2 * b_):
        b, p = core // 2, core % 2
        xb = x[b].reshape(nb, P, d_)
        xperm = np.concatenate([xb[p::2], xb[1 - p::2]], axis=0)
        xT = xperm.reshape(t_, d_).T  # [D, T]
        hi, lo = _split_fp8(np.ascontiguousarray(xT))
        # xf [n_pair*P, 4*T]: per pair rows: [hi_2d | hi_2d+1 | lo_2d | lo_2d+1]
        xf = np.empty((n_pair, P, 4, t_), fp8)
        hic = hi.reshape(n_d, P, t_)
        loc = lo.reshape(n_d, P, t_)
        xf[:, :, 0, :] = hic[0::2]
        xf[:, :, 1, :] = hic[1::2]
        xf[:, :, 2, :] = loc[0::2]
        xf[:, :, 3, :] = loc[1::2]
        oth = ones if p == 0 else zeros
        mask2 = np.concatenate([tri, oth], axis=1)
        in_maps.append({
            "xf": np.ascontiguousarray(xf.reshape(n_pair * P, 4 * t_)),
            "wqf": wq_f, "wkf": wk_f, "wvf": wv_f,
            "mask2": np.ascontiguousarray(mask2),
            "mask8": np.ascontiguousarray(mask2.astype(fp8)),
        })
    return in_maps


def gather_out(results, b_=B, t_=T, h_=H):
    """Combine per-core unnormalized partials into the full output."""
    nb = t_ // P
    acc = np.zeros((b_, nb, P, h_ + 1), dtype=np.float64)
    for core in range(2 * b_):
        b, p = core // 2, core % 2
        # out [P, nb*(h+1)] -> [nb, P, h+1] (block-major cols, partition=row)
        o = results[core]["out"].astype(np.float64)
        o = o.reshape(P, nb, h_ + 1).transpose(1, 0, 2)
        blocks = np.concatenate([np.arange(p, nb, 2),
                                 np.arange(1 - p, nb, 2)])
        acc[b, blocks] += o
    out = acc[..., :h_] / (acc[..., h_:h_ + 1] * WSCALE_V)
    return out.reshape(b_, t_, h_).astype(np.float32)


def kernel(x, Wq, Wk, Wv):
    from concourse.bass_utils import run_bass_kernel_spmd

    nc = _get_nc(D, T, H)
    in_maps = make_in_maps(x, Wq, Wk, Wv)
    res = run_bass_kernel_spmd(nc, in_maps, core_ids=list(range(N_CORES)))
    return gather_out(res.results)
